# revision 1
# baseline (speedup 1.0000x reference)
"""RSNA loss kernel for Trainium2, SPMD across 8 NeuronCores.

Strategy (data-parallel over batch):
  - Shard B=128 exams -> 16 per core.
  - Per exam, view pred/label [8192, 10] as SBUF tile [128 part, 640]
    (partition p holds l in [64p, 64p+64), free index j*10+c, j=l%64).
  - The seq_len mask over (p, j) is rank-2:
        mask[p,j] = a[p]*s[j] + b[p]*t[j]
    with a=[p <= len//64], b=[p < len//64], s=[j < len%64], t=1-s.
    So masked channel sums become TWO TensorE matmuls per exam
    (contract partitions with lhsT columns a/b), followed by a
    j-weighted fold (multiply by host-built s/t patterns + reduce).
  - Image BCE: log(p0), log(1-p0) on ScalarE (strided channel-0 slice),
    bce = y0*(lp-lq)+lq on VectorE, masked-summed by the same a/b
    matmul + s/t fold trick.
  - Device outputs per core: [32, 21] partial sums; host does the tiny
    final combine (exam-level BCE on [128,9], scalar reduction) in f64.
All mask tensors are tiny host-built inputs derived from seq_lens.
"""
import numpy as np
from contextlib import ExitStack

import concourse.bass as bass
import concourse.bacc as bacc
import concourse.tile as tile
from concourse import mybir
from concourse.bass_utils import run_bass_kernel_spmd

N_CORES = 8
B, L, C = 128, 8192, 10
EPC = B // N_CORES          # exams per core = 16
JP = 64                     # l's per partition
NP = 128                    # partitions
COLS = JP * C               # 640 free columns per exam
BCEW = EPC * JP             # 1024 bce columns (16 exams x 64)

IMAGE_WEIGHT = 0.0736196319
EXAM_WEIGHTS = np.array([0.0736196319, 0.09202453988, 0.1042944785, 0.1042944785,
                         0.1877300613, 0.06257668712, 0.06257668712, 0.2346625767,
                         0.0782208589], dtype=np.float64)

_NC_CACHE = {}


def build_nc():
    nc = bacc.Bacc(trn_type="TRN2")
    f32 = mybir.dt.float32
    pred = nc.declare_dram_parameter("pred", [NP, EPC, COLS], f32, isOutput=False)
    label = nc.declare_dram_parameter("label", [NP, EPC, COLS], f32, isOutput=False)
    bf16 = mybir.dt.bfloat16
    lhst = nc.declare_dram_parameter("lhst", [NP, EPC, 2 * EPC], bf16, isOutput=False)
    lhstd = nc.declare_dram_parameter("lhstd", [NP, 2 * EPC], f32, isOutput=False)
    sstt = nc.declare_dram_parameter("sstt", [2 * EPC, COLS], f32, isOutput=False)
    ssttb = nc.declare_dram_parameter("ssttb", [2 * EPC, BCEW], f32, isOutput=False)
    out = nc.declare_dram_parameter("out", [2 * EPC, 21], f32, isOutput=True)

    with tile.TileContext(nc) as tc, ExitStack() as ctx:
        consts = ctx.enter_context(tc.tile_pool(name="consts", bufs=1))
        data = ctx.enter_context(tc.tile_pool(name="data", bufs=4))
        conv = ctx.enter_context(tc.tile_pool(name="conv", bufs=2))
        bcep = ctx.enter_context(tc.tile_pool(name="bcep", bufs=1))
        psum = ctx.enter_context(tc.tile_pool(name="psum", bufs=1, space="PSUM"))
        post = ctx.enter_context(tc.tile_pool(name="post", bufs=1))

        # constants
        t_lhst = consts.tile([NP, EPC, 2 * EPC], bf16, tag="lhst")
        nc.sync.dma_start(out=t_lhst, in_=lhst[:, :, :])
        t_lhstd = consts.tile([NP, 2 * EPC], f32, tag="lhstd")
        nc.sync.dma_start(out=t_lhstd, in_=lhstd[:, :])
        t_sstt = consts.tile([2 * EPC, COLS], f32, tag="sstt")
        nc.sync.dma_start(out=t_sstt, in_=sstt[:, :])
        t_ssttb = consts.tile([2 * EPC, BCEW], f32, tag="ssttb")
        nc.sync.dma_start(out=t_ssttb, in_=ssttb[:, :])

        # batched channel-0 tiles
        LP = bcep.tile([NP, BCEW], f32, tag="LP")
        LQ = bcep.tile([NP, BCEW], f32, tag="LQ")
        Y0 = bcep.tile([NP, BCEW], f32, tag="Y0")
        T1 = bcep.tile([NP, BCEW], f32, tag="T1")
        BCE = bcep.tile([NP, BCEW], f32, tag="BCE")

        # psum accumulators
        P0p = psum.tile([2 * EPC, 320], f32, tag="P0p")
        P1p = psum.tile([2 * EPC, 320], f32, tag="P1p")
        P0l = psum.tile([2 * EPC, 320], f32, tag="P0l")
        P1l = psum.tile([2 * EPC, 320], f32, tag="P1l")
        PB0 = psum.tile([2 * EPC, 512], f32, tag="PB0")
        PB1 = psum.tile([2 * EPC, 512], f32, tag="PB1")

        GRP = 4                      # exams per DMA
        for tI in range(EPC // GRP):
            # one DMA per tensor loads GRP exams with 10KB-contiguous
            # per-partition runs; pred on the SP HWDGE ring, label on ACT's
            Tp = data.tile([NP, GRP, COLS], f32, tag="Tp")
            nc.sync.dma_start(
                out=Tp, in_=pred[:, GRP * tI:GRP * (tI + 1), :])
            Tl = data.tile([NP, GRP, COLS], f32, tag="Tl")
            nc.gpsimd.dma_start(
                out=Tl, in_=label[:, GRP * tI:GRP * (tI + 1), :])
            # bf16 copies for the TensorE path (2x matmul throughput);
            # pred on DVE, label on ACT — both have slack
            TpB = conv.tile([NP, GRP, COLS], bf16, tag="TpB")
            nc.vector.tensor_copy(TpB, Tp)
            TlB = conv.tile([NP, GRP, COLS], bf16, tag="TlB")
            nc.scalar.copy(TlB, Tl)
            for eo in range(GRP):
                e = GRP * tI + eo
                lhsT_e = t_lhst[:, e, :]
                st = dict(start=(e == 0), stop=(e == EPC - 1))
                nc.tensor.matmul(P0p, lhsT_e, TpB[:, eo, 0:320], **st)
                nc.tensor.matmul(P1p, lhsT_e, TpB[:, eo, 320:640], **st)
                nc.tensor.matmul(P0l, lhsT_e, TlB[:, eo, 0:320], **st)
                nc.tensor.matmul(P1l, lhsT_e, TlB[:, eo, 320:640], **st)
                # channel-0 strided views [128, 64]
                p0 = Tp[:, eo].rearrange("p (j c) -> p j c", c=C)[:, :, 0]
                y0 = Tl[:, eo].rearrange("p (j c) -> p j c", c=C)[:, :, 0]
                sl = slice(JP * e, JP * (e + 1))
                nc.scalar.activation(
                    out=LP[:, sl], in_=p0,
                    func=mybir.ActivationFunctionType.Ln)
                nc.scalar.activation(
                    out=LQ[:, sl], in_=p0,
                    func=mybir.ActivationFunctionType.Ln, bias=1.0, scale=-1.0)
                nc.vector.tensor_copy(Y0[:, sl], y0)

        # bce = y0*(lp - lq) + lq
        nc.vector.tensor_sub(T1, LP, LQ)
        nc.vector.tensor_mul(T1, T1, Y0)
        nc.vector.tensor_add(BCE, T1, LQ)
        nc.tensor.matmul(PB0, t_lhstd, BCE[:, 0:512], start=True, stop=True)
        nc.tensor.matmul(PB1, t_lhstd, BCE[:, 512:1024], start=True, stop=True)

        # post: weight by s/t patterns, fold j, emit [32, 21]
        O = post.tile([2 * EPC, 21], f32, tag="O")
        W0 = post.tile([2 * EPC, 320], f32, tag="W0")
        W1 = post.tile([2 * EPC, 320], f32, tag="W1")
        for (Pa, Pb, oc) in ((P0p, P1p, 0), (P0l, P1l, 10)):
            nc.vector.tensor_mul(W0, Pa, t_sstt[:, 0:320])
            nc.vector.tensor_mul(W1, Pb, t_sstt[:, 320:640])
            nc.vector.tensor_add(W0, W0, W1)
            nc.vector.tensor_reduce(
                out=O[:, oc:oc + 10],
                in_=W0.rearrange("p (j c) -> p c j", c=C),
                axis=mybir.AxisListType.X, op=mybir.AluOpType.add)
        WB0 = post.tile([2 * EPC, 512], f32, tag="WB0")
        WB1 = post.tile([2 * EPC, 512], f32, tag="WB1")
        nc.vector.tensor_mul(WB0, PB0, t_ssttb[:, 0:512])
        nc.vector.tensor_mul(WB1, PB1, t_ssttb[:, 512:1024])
        nc.vector.tensor_add(WB0, WB0, WB1)
        nc.vector.tensor_reduce(
            out=O[:, 20:21], in_=WB0,
            axis=mybir.AxisListType.X, op=mybir.AluOpType.add)
        nc.sync.dma_start(out=out[:, :], in_=O)
    nc.finalize()
    return nc


def _mask_tensors(lens):
    """Per-core mask inputs from the 16 seq_lens of this core."""
    lhst = np.zeros((EPC, NP, 2 * EPC), np.float32)
    lhstd = np.zeros((NP, 2 * EPC), np.float32)
    sstt = np.zeros((2 * EPC, COLS), np.float32)
    ssttb = np.zeros((2 * EPC, BCEW), np.float32)
    p_idx = np.arange(NP)
    j_idx = np.arange(JP)
    for e, ln in enumerate(lens):
        P, r = divmod(int(ln), JP)
        a = (p_idx <= P).astype(np.float32)
        b = (p_idx < P).astype(np.float32)
        s = (j_idx < r).astype(np.float32)
        t = 1.0 - s
        lhst[e, :, 2 * e] = a
        lhst[e, :, 2 * e + 1] = b
        lhstd[:, 2 * e] = a
        lhstd[:, 2 * e + 1] = b
        sstt[2 * e, :] = np.repeat(s, C)
        sstt[2 * e + 1, :] = np.repeat(t, C)
        ssttb[2 * e, JP * e:JP * (e + 1)] = s
        ssttb[2 * e + 1, JP * e:JP * (e + 1)] = t
    bf16np = mybir.dt.np(mybir.dt.bfloat16)
    return (np.ascontiguousarray(lhst.transpose(1, 0, 2)).astype(bf16np),
            lhstd, sstt, ssttb)


def make_in_maps(pred, label, seq_lens):
    in_maps = []
    for i in range(N_CORES):
        sl = slice(i * EPC, (i + 1) * EPC)
        lhst, lhstd, sstt, ssttb = _mask_tensors(seq_lens[sl])
        in_maps.append({
            "pred": np.ascontiguousarray(
                pred[sl].reshape(EPC, NP, COLS).transpose(1, 0, 2)),
            "label": np.ascontiguousarray(
                label[sl].reshape(EPC, NP, COLS).transpose(1, 0, 2)),
            "lhst": lhst, "lhstd": lhstd, "sstt": sstt, "ssttb": ssttb,
        })
    return in_maps


def finish(outs, seq_lens):
    """Host-side final combine from the 8 per-core [32, 21] outputs."""
    w = EXAM_WEIGHTS
    exam_loss = 0.0
    image_loss = 0.0
    tw_img = 0.0
    for i in range(N_CORES):
        O = outs[i].astype(np.float64)
        S = O[0::2] + O[1::2]                 # [16, 21] a+b parts
        lens = seq_lens[i * EPC:(i + 1) * EPC].astype(np.float64)
        pm = S[:, 1:10] / lens[:, None]
        ym = S[:, 11:20] / lens[:, None]
        exam_bce = -(ym * np.log(pm) + (1.0 - ym) * np.log(1.0 - pm))
        exam_loss += float(np.sum(exam_bce * w[None, :]))
        y0m = S[:, 10] / lens
        imgw = IMAGE_WEIGHT * y0m
        image_loss += float(np.sum(-S[:, 20] * imgw))
        tw_img += float(np.sum(imgw * lens))
    total_weights = B * float(np.sum(w)) + tw_img
    return np.float32((exam_loss + image_loss) / total_weights)


def kernel(pred, label, seq_lens):
    if "nc" not in _NC_CACHE:
        _NC_CACHE["nc"] = build_nc()
    nc = _NC_CACHE["nc"]
    in_maps = make_in_maps(np.asarray(pred), np.asarray(label),
                           np.asarray(seq_lens))
    res = run_bass_kernel_spmd(nc, in_maps, core_ids=list(range(N_CORES)))
    outs = [res.results[i]["out"] for i in range(N_CORES)]
    return finish(outs, np.asarray(seq_lens))


if __name__ == "__main__":
    rng = np.random.default_rng(0)
    pred = (rng.random((B, L, C), np.float32) * 0.98 + 0.01).astype(np.float32)
    label = (rng.random((B, L, C), np.float32) * 0.98 + 0.01).astype(np.float32)
    seq_lens = rng.integers(1, L + 1, size=(B,)).astype(np.int32)
    got = kernel(pred=pred, label=label, seq_lens=seq_lens)
    print("kernel:", got)



# revision 8
# speedup vs baseline: 1.5320x; 1.5320x over previous
"""RSNA loss kernel for Trainium2, SPMD across 8 NeuronCores.

Strategy (data-parallel over batch):
  - Shard B=128 exams -> 16 per core.
  - Host pre-splits each [8192, 10] exam into channels 1-9 (exam path)
    and channel 0 (image path), converts to bf16 (rel tol is 2e-2;
    bf16 end-to-end error is ~3e-5) and lays out partition-major:
      pc19/lc19 [128, 16, 576]  (partition p holds l in [64p, 64p+64))
      p0/y0     [128, 1024]     (col = 64*e + j)
    This halves HBM traffic vs f32 and makes channel-0 reads contiguous.
  - The seq_len mask over (p, j) is rank-2:
        mask[p,j] = a[p]*s[j] + b[p]*t[j]
    with a=[p <= len//64], b=[p < len//64], s=[j < len%64], t=1-s.
    Masked channel sums = per-exam TensorE matmuls with lhsT [128, 2]
    (a,b columns) into psum rows [2e:2e+2], then a host-built s/t
    weighted fold over j.
  - Image BCE: Ln(p0), Ln(1-p0) on ScalarE (contiguous), bce =
    y0*(lp-lq)+lq on VectorE, masked-summed by a diag a/b matmul +
    s/t fold; same fold gives the masked y0 sums.
  - All loads ride the two hardware-DGE rings (sync + scalar engines),
    kicked up-front into dedicated tiles so DMA never stalls.
  - Device outputs per core: [32, 20] partial sums; host does the tiny
    final combine in f64.
"""
import numpy as np
from contextlib import ExitStack

import concourse.bass as bass
import concourse.bacc as bacc
import concourse.tile as tile
from concourse import mybir
from concourse.bass_utils import run_bass_kernel_spmd

N_CORES = 8
B, L, C = 128, 8192, 10
EPC = B // N_CORES          # exams per core = 16
JP = 64                     # l's per partition
NP = 128                    # partitions
C9 = C - 1                  # exam-path channels
CW = JP * C9                # 576 free columns per exam (ch 1-9)
HW = CW // 2                # 288, fits the 512 moving-free-dim limit
BCEW = EPC * JP             # 1024 channel-0 columns (16 exams x 64)
GRP = 4                     # exams per DMA / matmul group
NG = EPC // GRP
NOUT = 2 * C9 + 2           # 20 output columns

IMAGE_WEIGHT = 0.0736196319
EXAM_WEIGHTS = np.array([0.0736196319, 0.09202453988, 0.1042944785, 0.1042944785,
                         0.1877300613, 0.06257668712, 0.06257668712, 0.2346625767,
                         0.0782208589], dtype=np.float64)

_NC_CACHE = {}


def build_nc():
    nc = bacc.Bacc(trn_type="TRN2")
    f32 = mybir.dt.float32
    bf16 = mybir.dt.bfloat16
    pc19 = nc.declare_dram_parameter("pc19", [NP, EPC, CW], bf16, isOutput=False)
    lc19 = nc.declare_dram_parameter("lc19", [NP, EPC, CW], bf16, isOutput=False)
    p0 = nc.declare_dram_parameter("p0", [NP, BCEW], bf16, isOutput=False)
    y0 = nc.declare_dram_parameter("y0", [NP, BCEW], bf16, isOutput=False)
    lhst = nc.declare_dram_parameter("lhst", [NP, EPC, 2 * EPC], bf16,
                                     isOutput=False)
    lhstd = nc.declare_dram_parameter("lhstd", [NP, 2 * EPC], bf16, isOutput=False)
    sstt = nc.declare_dram_parameter("sstt", [2 * EPC, CW], f32, isOutput=False)
    ssttb = nc.declare_dram_parameter("ssttb", [2 * EPC, BCEW], f32, isOutput=False)
    out = nc.declare_dram_parameter("out", [2 * EPC, NOUT], f32, isOutput=True)

    with tile.TileContext(nc) as tc, ExitStack() as ctx:
        pool = ctx.enter_context(tc.tile_pool(name="main", bufs=1))
        psum = ctx.enter_context(tc.tile_pool(name="psum", bufs=1, space="PSUM"))

        # dedicated tiles (no recycling -> every DMA can be in flight)
        t_lhst = pool.tile([NP, EPC, 2 * EPC], bf16, tag="lhst")
        t_lhstd = pool.tile([NP, 2 * EPC], bf16, tag="lhstd")
        t_sstt = pool.tile([2 * EPC, CW], f32, tag="sstt")
        t_ssttb = pool.tile([2 * EPC, BCEW], f32, tag="ssttb")
        t_p0 = pool.tile([NP, BCEW], bf16, tag="p0")
        t_y0 = pool.tile([NP, BCEW], bf16, tag="y0")
        Tp = [pool.tile([NP, GRP, CW], bf16, tag=f"Tp{g}", name=f"Tp{g}")
              for g in range(NG)]
        Tl = [pool.tile([NP, GRP, CW], bf16, tag=f"Tl{g}", name=f"Tl{g}")
              for g in range(NG)]
        LP = pool.tile([NP, BCEW], bf16, tag="LP")
        LQ = pool.tile([NP, BCEW], bf16, tag="LQ")
        T1 = pool.tile([NP, BCEW], bf16, tag="T1")
        BCE = pool.tile([NP, BCEW], bf16, tag="BCE")
        W = pool.tile([2 * EPC, CW], f32, tag="W")
        WB = pool.tile([2 * EPC, BCEW], f32, tag="WB")
        O = pool.tile([2 * EPC, NOUT], f32, tag="O")

        # ring A (sync HWDGE): small consts first, then pred groups
        nc.sync.dma_start(out=t_lhst, in_=lhst[:, :, :])
        nc.sync.dma_start(out=t_lhstd, in_=lhstd[:, :])
        nc.sync.dma_start(out=t_p0, in_=p0[:, :])
        for g in range(NG):
            nc.sync.dma_start(out=Tp[g], in_=pc19[:, GRP * g:GRP * (g + 1), :])
        nc.sync.dma_start(out=t_sstt, in_=sstt[:, :])

        # ring B (scalar HWDGE): y0 first, then label groups
        nc.scalar.dma_start(out=t_y0, in_=y0[:, :])
        for g in range(NG):
            nc.scalar.dma_start(out=Tl[g], in_=lc19[:, GRP * g:GRP * (g + 1), :])
        nc.scalar.dma_start(out=t_ssttb, in_=ssttb[:, :])

        # image-path logs on ScalarE (contiguous bf16)
        nc.scalar.activation(out=LP, in_=t_p0,
                             func=mybir.ActivationFunctionType.Ln)
        nc.scalar.activation(out=LQ, in_=t_p0,
                             func=mybir.ActivationFunctionType.Ln,
                             bias=1.0, scale=-1.0)
        # bce = y0*(lp - lq) + lq  (= -image BCE)
        nc.vector.tensor_sub(T1, LP, LQ)
        nc.vector.tensor_mul(T1, T1, t_y0)
        nc.vector.tensor_add(BCE, T1, LQ)

        # psum accumulators
        Pp0 = psum.tile([2 * EPC, HW], f32, tag="Pp0")
        Pp1 = psum.tile([2 * EPC, HW], f32, tag="Pp1")
        Pl0 = psum.tile([2 * EPC, HW], f32, tag="Pl0")
        Pl1 = psum.tile([2 * EPC, HW], f32, tag="Pl1")
        PY0 = psum.tile([2 * EPC, 512], f32, tag="PY0")
        PY1 = psum.tile([2 * EPC, 512], f32, tag="PY1")
        PB0 = psum.tile([2 * EPC, 512], f32, tag="PB0")
        PB1 = psum.tile([2 * EPC, 512], f32, tag="PB1")

        for g in range(NG):
            for eo in range(GRP):
                e = GRP * g + eo
                lhsT_e = t_lhst[:, e, :]
                st = dict(start=(e == 0), stop=(e == EPC - 1))
                nc.tensor.matmul(Pp0, lhsT_e, Tp[g][:, eo, 0:HW], **st)
                nc.tensor.matmul(Pp1, lhsT_e, Tp[g][:, eo, HW:CW], **st)
                nc.tensor.matmul(Pl0, lhsT_e, Tl[g][:, eo, 0:HW], **st)
                nc.tensor.matmul(Pl1, lhsT_e, Tl[g][:, eo, HW:CW], **st)
            if g == 0:
                # masked y0 sums (y0 resident long before group 1 lands)
                nc.tensor.matmul(PY0, t_lhstd, t_y0[:, 0:512])
                nc.tensor.matmul(PY1, t_lhstd, t_y0[:, 512:BCEW])
            if g == 2:
                # image bce sums (bce chain done well before group 3)
                nc.tensor.matmul(PB0, t_lhstd, BCE[:, 0:512])
                nc.tensor.matmul(PB1, t_lhstd, BCE[:, 512:BCEW])

        # post: weight by s/t patterns, fold j, emit [32, 20]
        for (Pa, Pb, oc) in ((Pp0, Pp1, 0), (Pl0, Pl1, C9)):
            nc.vector.tensor_mul(W[:, 0:HW], Pa, t_sstt[:, 0:HW])
            nc.vector.tensor_mul(W[:, HW:CW], Pb, t_sstt[:, HW:CW])
            nc.vector.tensor_reduce(
                out=O[:, oc:oc + C9],
                in_=W.rearrange("p (j c) -> p c j", c=C9),
                axis=mybir.AxisListType.X, op=mybir.AluOpType.add)
        for (Pa, Pb, oc) in ((PY0, PY1, 2 * C9), (PB0, PB1, 2 * C9 + 1)):
            nc.vector.tensor_mul(WB[:, 0:512], Pa, t_ssttb[:, 0:512])
            nc.vector.tensor_mul(WB[:, 512:BCEW], Pb, t_ssttb[:, 512:BCEW])
            nc.vector.tensor_reduce(
                out=O[:, oc:oc + 1], in_=WB,
                axis=mybir.AxisListType.X, op=mybir.AluOpType.add)
        nc.sync.dma_start(out=out[:, :], in_=O)
    nc.finalize()
    return nc


def _mask_tensors(lens):
    """Per-core mask inputs from the 16 seq_lens of this core."""
    lhst = np.zeros((NP, EPC, 2 * EPC), np.float32)
    lhstd = np.zeros((NP, 2 * EPC), np.float32)
    sstt = np.zeros((2 * EPC, CW), np.float32)
    ssttb = np.zeros((2 * EPC, BCEW), np.float32)
    p_idx = np.arange(NP)
    j_idx = np.arange(JP)
    for e, ln in enumerate(lens):
        P, r = divmod(int(ln), JP)
        a = (p_idx <= P).astype(np.float32)
        b = (p_idx < P).astype(np.float32)
        s = (j_idx < r).astype(np.float32)
        t = 1.0 - s
        lhst[:, e, 2 * e] = a
        lhst[:, e, 2 * e + 1] = b
        lhstd[:, 2 * e] = a
        lhstd[:, 2 * e + 1] = b
        sstt[2 * e, :] = np.repeat(s, C9)
        sstt[2 * e + 1, :] = np.repeat(t, C9)
        ssttb[2 * e, JP * e:JP * (e + 1)] = s
        ssttb[2 * e + 1, JP * e:JP * (e + 1)] = t
    bf16np = mybir.dt.np(mybir.dt.bfloat16)
    return lhst.astype(bf16np), lhstd.astype(bf16np), sstt, ssttb


def make_in_maps(pred, label, seq_lens):
    bf16np = mybir.dt.np(mybir.dt.bfloat16)
    in_maps = []
    for i in range(N_CORES):
        sl = slice(i * EPC, (i + 1) * EPC)
        lhst, lhstd, sstt, ssttb = _mask_tensors(seq_lens[sl])
        m = {"lhst": lhst, "lhstd": lhstd, "sstt": sstt, "ssttb": ssttb}
        for name0, name19, t in (("p0", "pc19", pred), ("y0", "lc19", label)):
            r = t[sl].reshape(EPC, NP, JP, C)
            m[name19] = np.ascontiguousarray(
                r[..., 1:].transpose(1, 0, 2, 3)).reshape(
                    NP, EPC, CW).astype(bf16np)
            m[name0] = np.ascontiguousarray(
                r[..., 0].transpose(1, 0, 2)).reshape(
                    NP, BCEW).astype(bf16np)
        in_maps.append(m)
    return in_maps


def finish(outs, seq_lens):
    """Host-side final combine from the 8 per-core [32, 20] outputs."""
    w = EXAM_WEIGHTS
    exam_loss = 0.0
    image_loss = 0.0
    tw_img = 0.0
    for i in range(N_CORES):
        O = outs[i].astype(np.float64)
        S = O[0::2] + O[1::2]                 # [16, 20] a+b parts
        lens = seq_lens[i * EPC:(i + 1) * EPC].astype(np.float64)
        pm = S[:, 0:C9] / lens[:, None]
        ym = S[:, C9:2 * C9] / lens[:, None]
        exam_bce = -(ym * np.log(pm) + (1.0 - ym) * np.log(1.0 - pm))
        exam_loss += float(np.sum(exam_bce * w[None, :]))
        y0m = S[:, 2 * C9] / lens
        imgw = IMAGE_WEIGHT * y0m
        image_loss += float(np.sum(-S[:, 2 * C9 + 1] * imgw))
        tw_img += float(np.sum(imgw * lens))
    total_weights = B * float(np.sum(w)) + tw_img
    return np.float32((exam_loss + image_loss) / total_weights)


def kernel(pred, label, seq_lens):
    if "nc" not in _NC_CACHE:
        _NC_CACHE["nc"] = build_nc()
    nc = _NC_CACHE["nc"]
    in_maps = make_in_maps(np.asarray(pred), np.asarray(label),
                           np.asarray(seq_lens))
    res = run_bass_kernel_spmd(nc, in_maps, core_ids=list(range(N_CORES)))
    outs = [res.results[i]["out"] for i in range(N_CORES)]
    return finish(outs, np.asarray(seq_lens))


if __name__ == "__main__":
    rng = np.random.default_rng(0)
    pred = (rng.random((B, L, C), np.float32) * 0.98 + 0.01).astype(np.float32)
    label = (rng.random((B, L, C), np.float32) * 0.98 + 0.01).astype(np.float32)
    seq_lens = rng.integers(1, L + 1, size=(B,)).astype(np.int32)
    got = kernel(pred=pred, label=label, seq_lens=seq_lens)
    print("kernel:", got)


# revision 10
# speedup vs baseline: 1.8224x; 1.1896x over previous
"""RSNA loss kernel for Trainium2, SPMD across 8 NeuronCores.

Strategy (data-parallel over batch):
  - Shard B=128 exams -> 16 per core.
  - Host pre-splits each [8192, 10] exam into channels 1-9 + channel 0,
    converts to bf16 (rel tol is 2e-2; bf16 end-to-end error ~3e-5) and
    lays out partition-major (partition p holds l in [64p, 64p+64)):
      pc19 [128, 16, 576]   pred ch 1-9, free = (j, c)
      lc19y [128, 16, 640]  label ch 1-9 (cols 0:576) + label ch 0 (y0,
                            cols 576:640) so masked y0 sums ride the
                            same per-exam matmuls
      p0   [128, 1024]      pred ch 0 (col = 64*e + j)
  - The seq_len mask over (p, j) is rank-2:
        mask[p,j] = a[p]*s[j] + b[p]*t[j]
    with a=[p <= len//64], b=[p < len//64], s=[j < len%64], t=1-s.
    Per exam, two matmuls vs a/b columns (lhsT [128,32], cols 2e/2e+1)
    accumulate partition sums into psum rows 2e/2e+1; the tiny s/t
    weighted fold over j happens on the HOST from the raw psum dump.
  - Image BCE: Ln(p0), Ln(1-p0) on ScalarE (contiguous), bce =
    y0*(lp-lq)+lq on VectorE per label group, masked-summed by a diag
    a/b matmul; host folds s/t.
  - All loads ride the two hardware-DGE rings (sync + scalar engines),
    kicked up-front into dedicated tiles so DMA never stalls. Groups
    taper (5,5,4,2) so little PE work remains after the last DMA.
  - Device outputs per core: raw psum [32, 2240]; host does the tiny
    final combine in f64.
"""
import numpy as np
from contextlib import ExitStack

import concourse.bass as bass
import concourse.bacc as bacc
import concourse.tile as tile
from concourse import mybir
from concourse.bass_utils import run_bass_kernel_spmd

N_CORES = 8
B, L, C = 128, 8192, 10
EPC = B // N_CORES          # exams per core = 16
JP = 64                     # l's per partition
NP = 128                    # partitions
C9 = C - 1                  # exam-path channels
CW = JP * C9                # 576 ch1-9 columns per exam
LW = CW + JP                # 640 label columns per exam (ch1-9 + y0)
BCEW = EPC * JP             # 1024 channel-0 columns (16 exams x 64)
GROUPS = [5, 5, 4, 2]       # exams per DMA / matmul group (tapered)
STARTS = [0, 5, 10, 14]
MW = 2 * EPC                # 32 mask columns (a/b per exam)
OUTW = CW + LW + BCEW       # 2240 output columns

IMAGE_WEIGHT = 0.0736196319
EXAM_WEIGHTS = np.array([0.0736196319, 0.09202453988, 0.1042944785, 0.1042944785,
                         0.1877300613, 0.06257668712, 0.06257668712, 0.2346625767,
                         0.0782208589], dtype=np.float64)

_NC_CACHE = {}


def build_nc():
    nc = bacc.Bacc(trn_type="TRN2")
    f32 = mybir.dt.float32
    bf16 = mybir.dt.bfloat16
    pc19 = nc.declare_dram_parameter("pc19", [NP, EPC, CW], bf16, isOutput=False)
    lc19y = nc.declare_dram_parameter("lc19y", [NP, EPC, LW], bf16,
                                      isOutput=False)
    p0 = nc.declare_dram_parameter("p0", [NP, BCEW], bf16, isOutput=False)
    # masks: [lhst (16 exams x 32) | lhstd (32)] = 544 bf16 columns
    masks = nc.declare_dram_parameter("masks", [NP, EPC * MW + MW], bf16,
                                      isOutput=False)
    out = nc.declare_dram_parameter("out", [MW, OUTW], f32, isOutput=True)

    with tile.TileContext(nc) as tc, ExitStack() as ctx:
        pool = ctx.enter_context(tc.tile_pool(name="main", bufs=1))
        psum = ctx.enter_context(tc.tile_pool(name="psum", bufs=1, space="PSUM"))

        # dedicated tiles (no recycling -> every DMA can be in flight)
        t_p0 = pool.tile([NP, BCEW], bf16, tag="p0")
        t_masks = pool.tile([NP, EPC * MW + MW], bf16, tag="masks")
        Tp = [pool.tile([NP, n, CW], bf16, tag=f"Tp{g}", name=f"Tp{g}")
              for g, n in enumerate(GROUPS)]
        Tl = [pool.tile([NP, n, LW], bf16, tag=f"Tl{g}", name=f"Tl{g}")
              for g, n in enumerate(GROUPS)]
        LP = pool.tile([NP, BCEW], bf16, tag="LP")
        LQ = pool.tile([NP, BCEW], bf16, tag="LQ")
        T1 = pool.tile([NP, BCEW], bf16, tag="T1")
        BCE = pool.tile([NP, BCEW], bf16, tag="BCE")
        OUT = pool.tile([MW, OUTW], f32, tag="OUT")

        # ring A (sync HWDGE): p0 + masks first, then pred groups
        nc.sync.dma_start(out=t_p0, in_=p0[:, :])
        nc.sync.dma_start(out=t_masks, in_=masks[:, :])
        for g, n in enumerate(GROUPS):
            s = STARTS[g]
            nc.sync.dma_start(out=Tp[g], in_=pc19[:, s:s + n, :])
        # ring B (scalar HWDGE): label groups
        for g, n in enumerate(GROUPS):
            s = STARTS[g]
            nc.scalar.dma_start(out=Tl[g], in_=lc19y[:, s:s + n, :])

        # image-path logs on ScalarE (contiguous bf16, ready early)
        nc.scalar.activation(out=LP, in_=t_p0,
                             func=mybir.ActivationFunctionType.Ln)
        nc.scalar.activation(out=LQ, in_=t_p0,
                             func=mybir.ActivationFunctionType.Ln,
                             bias=1.0, scale=-1.0)
        # bce = y0*(lp - lq) + lq  (= -image BCE), per label group
        for g, n in enumerate(GROUPS):
            s = STARTS[g]
            cs = slice(JP * s, JP * (s + n))
            y0v = Tl[g][:, :, CW:LW]
            sh = dict(e=n, j=JP)
            nc.vector.tensor_sub(T1[:, cs], LP[:, cs], LQ[:, cs])
            nc.vector.tensor_mul(
                T1[:, cs].rearrange("p (e j) -> p e j", **sh),
                T1[:, cs].rearrange("p (e j) -> p e j", **sh), y0v)
            nc.vector.tensor_add(BCE[:, cs], T1[:, cs], LQ[:, cs])

        # psum accumulators
        Pp0 = psum.tile([MW, CW // 2], f32, tag="Pp0")
        Pp1 = psum.tile([MW, CW // 2], f32, tag="Pp1")
        Pl0 = psum.tile([MW, LW // 2], f32, tag="Pl0")
        Pl1 = psum.tile([MW, LW // 2], f32, tag="Pl1")
        PB0 = psum.tile([MW, BCEW // 2], f32, tag="PB0")
        PB1 = psum.tile([MW, BCEW // 2], f32, tag="PB1")

        for g, n in enumerate(GROUPS):
            for eo in range(n):
                e = STARTS[g] + eo
                lhsT_e = t_masks[:, MW * e:MW * (e + 1)]
                st = dict(start=(e == 0), stop=(e == EPC - 1))
                nc.tensor.matmul(Pp0, lhsT_e, Tp[g][:, eo, 0:CW // 2], **st)
                nc.tensor.matmul(Pp1, lhsT_e, Tp[g][:, eo, CW // 2:CW], **st)
                nc.tensor.matmul(Pl0, lhsT_e, Tl[g][:, eo, 0:LW // 2], **st)
                nc.tensor.matmul(Pl1, lhsT_e, Tl[g][:, eo, LW // 2:LW], **st)
        lhstd = t_masks[:, EPC * MW:EPC * MW + MW]
        nc.tensor.matmul(PB0, lhstd, BCE[:, 0:BCEW // 2])
        nc.tensor.matmul(PB1, lhstd, BCE[:, BCEW // 2:BCEW])

        # raw psum -> SBUF (split across three engines), then DMA out
        c = 0
        segs = [(Pp0, CW // 2), (Pp1, CW // 2), (Pl0, LW // 2), (Pl1, LW // 2),
                (PB0, BCEW // 2), (PB1, BCEW // 2)]
        engs = [nc.scalar.copy, nc.scalar.copy,
                nc.vector.tensor_copy, nc.vector.tensor_copy,
                nc.scalar.copy, nc.vector.tensor_copy]
        for (P, wdt), cp in zip(segs, engs):
            cp(OUT[:, c:c + wdt], P)
            c += wdt
        nc.sync.dma_start(out=out[:, 0:CW + LW], in_=OUT[:, 0:CW + LW])
        nc.scalar.dma_start(out=out[:, CW + LW:OUTW], in_=OUT[:, CW + LW:OUTW])
    nc.finalize()
    return nc


def _mask_tensors(lens):
    """Per-core [128, 544] bf16 mask columns from this core's seq_lens."""
    m = np.zeros((NP, EPC * MW + MW), np.float32)
    p_idx = np.arange(NP)
    for e, ln in enumerate(lens):
        P = int(ln) // JP
        a = (p_idx <= P).astype(np.float32)
        b = (p_idx < P).astype(np.float32)
        m[:, MW * e + 2 * e] = a
        m[:, MW * e + 2 * e + 1] = b
        m[:, EPC * MW + 2 * e] = a
        m[:, EPC * MW + 2 * e + 1] = b
    return m.astype(mybir.dt.np(mybir.dt.bfloat16))


def make_in_maps(pred, label, seq_lens):
    bf16np = mybir.dt.np(mybir.dt.bfloat16)
    in_maps = []
    for i in range(N_CORES):
        sl = slice(i * EPC, (i + 1) * EPC)
        r = pred[sl].reshape(EPC, NP, JP, C)
        pc19 = np.ascontiguousarray(
            r[..., 1:].transpose(1, 0, 2, 3)).reshape(NP, EPC, CW)
        p0 = np.ascontiguousarray(
            r[..., 0].transpose(1, 0, 2)).reshape(NP, BCEW)
        rl = label[sl].reshape(EPC, NP, JP, C)
        lc19y = np.concatenate(
            [rl[..., 1:].reshape(EPC, NP, CW), rl[..., 0].reshape(EPC, NP, JP)],
            axis=2).transpose(1, 0, 2)
        in_maps.append({
            "pc19": pc19.astype(bf16np),
            "lc19y": np.ascontiguousarray(lc19y).astype(bf16np),
            "p0": p0.astype(bf16np),
            "masks": _mask_tensors(seq_lens[sl]),
        })
    return in_maps


def finish(outs, seq_lens):
    """Host-side s/t fold + final combine from the 8 [32, 2240] dumps."""
    w = EXAM_WEIGHTS
    j_idx = np.arange(JP)
    exam_loss = 0.0
    image_loss = 0.0
    tw_img = 0.0
    for i in range(N_CORES):
        O = outs[i].astype(np.float64)
        lens = seq_lens[i * EPC:(i + 1) * EPC].astype(np.float64)
        rv = (lens % JP).astype(np.int64)
        s = (j_idx[None, :] < rv[:, None]).astype(np.float64)   # [16, 64]
        t = 1.0 - s
        A, Bp = O[0::2], O[1::2]                                # [16, 2240]
        # pred ch1-9: cols 0:576 as (j, c)
        Pa = A[:, 0:CW].reshape(EPC, JP, C9)
        Pb = Bp[:, 0:CW].reshape(EPC, JP, C9)
        predsum = np.einsum('ej,ejc->ec', s, Pa) + np.einsum('ej,ejc->ec', t, Pb)
        # label ch1-9 + y0: cols 576:1216 as (j, c) then (j,)
        La = A[:, CW:CW + LW]
        Lb = Bp[:, CW:CW + LW]
        labsum = (np.einsum('ej,ejc->ec', s, La[:, 0:CW].reshape(EPC, JP, C9))
                  + np.einsum('ej,ejc->ec', t, Lb[:, 0:CW].reshape(EPC, JP, C9)))
        y0sum = np.sum(s * La[:, CW:LW], axis=1) + np.sum(t * Lb[:, CW:LW], axis=1)
        # bce: cols 1216:2240, exam e owns cols 64e:64e+64
        Ba = A[:, CW + LW:].reshape(EPC, EPC, JP)[np.arange(EPC), np.arange(EPC)]
        Bb = Bp[:, CW + LW:].reshape(EPC, EPC, JP)[np.arange(EPC), np.arange(EPC)]
        bcesum = np.sum(s * Ba, axis=1) + np.sum(t * Bb, axis=1)

        pm = predsum / lens[:, None]
        ym = labsum / lens[:, None]
        exam_bce = -(ym * np.log(pm) + (1.0 - ym) * np.log(1.0 - pm))
        exam_loss += float(np.sum(exam_bce * w[None, :]))
        y0m = y0sum / lens
        imgw = IMAGE_WEIGHT * y0m
        image_loss += float(np.sum(-bcesum * imgw))
        tw_img += float(np.sum(imgw * lens))
    total_weights = B * float(np.sum(w)) + tw_img
    return np.float32((exam_loss + image_loss) / total_weights)


def kernel(pred, label, seq_lens):
    if "nc" not in _NC_CACHE:
        _NC_CACHE["nc"] = build_nc()
    nc = _NC_CACHE["nc"]
    in_maps = make_in_maps(np.asarray(pred), np.asarray(label),
                           np.asarray(seq_lens))
    res = run_bass_kernel_spmd(nc, in_maps, core_ids=list(range(N_CORES)))
    outs = [res.results[i]["out"] for i in range(N_CORES)]
    return finish(outs, np.asarray(seq_lens))


if __name__ == "__main__":
    rng = np.random.default_rng(0)
    pred = (rng.random((B, L, C), np.float32) * 0.98 + 0.01).astype(np.float32)
    label = (rng.random((B, L, C), np.float32) * 0.98 + 0.01).astype(np.float32)
    seq_lens = rng.integers(1, L + 1, size=(B,)).astype(np.int32)
    got = kernel(pred=pred, label=label, seq_lens=seq_lens)
    print("kernel:", got)


# revision 11
# speedup vs baseline: 1.9754x; 1.0839x over previous
"""RSNA loss kernel for Trainium2, SPMD across 8 NeuronCores.

Strategy (data-parallel over batch):
  - Shard B=128 exams -> 16 per core.
  - Host pre-splits each [8192, 10] exam into channels 1-9 + channel 0
    and quantizes: ch1-9 and label-ch0 (y0) to fp8e4m3, pred-ch0 (p0)
    to bf16 (rel tol is 2e-2; end-to-end error ~2.4e-5, verified on the
    fixed inputs). Layout is partition-major; partition p holds
    l in [64p, 64p+64), split as two interleaved 32-blocks (i, j):
      pc19  [128, 16, 2, 288] fp8   pred ch1-9, free = (i, j, c)
      lc19y [128, 16, 2, 320] fp8   label ch1-9 (cols 0:288) + y0
                                    (cols 288:320) per i-half
      p0m   [128, 1056] bf16        pred ch0 (cols 0:1024, l-order)
                                    + diag a/b mask (cols 1024:1056)
      mask8 [128, 16, 2, 32] fp8    per-exam DoubleRow lhsT (a/b at
                                    cols 2e/2e+1, rows v = 2p+i)
  - The seq_len mask over virtual rows v = l//32 is rank-2:
        mask[v,j] = a[v]*s[j] + b[v]*t[j],  a=[v <= len//32] etc.
    One fp8 DoubleRow matmul per exam per tensor (contraction over
    256 virtual rows, 2x throughput) accumulates a/b-weighted sums
    into psum rows 2e/2e+1; the tiny s/t fold over j happens on the
    HOST from the raw psum dump.
  - Image BCE: Ln(p0), Ln(1-p0) on ScalarE, bce = y0*(lp-lq)+lq on
    VectorE per label group, masked-summed by a bf16 diag a/b matmul
    (64-block mask); host folds s/t.
  - All loads ride the two hardware-DGE rings (sync + scalar engines),
    kicked up-front into dedicated tiles. First group is small so the
    PE pipeline starts early; ring loads are balanced.
  - Device outputs per core: raw psum [32, 1632]; host does the tiny
    final combine in f64.
"""
import numpy as np
from contextlib import ExitStack

import concourse.bass as bass
import concourse.bacc as bacc
import concourse.tile as tile
from concourse import mybir
from concourse.bass_utils import run_bass_kernel_spmd

N_CORES = 8
B, L, C = 128, 8192, 10
EPC = B // N_CORES          # exams per core = 16
JP = 64                     # l's per partition
NP = 128                    # partitions
C9 = C - 1                  # exam-path channels
J2 = JP // 2                # 32 l's per virtual row
PW = J2 * C9                # 288 pred cols per exam (per i-half)
LWC = PW + J2               # 320 label cols per exam (ch1-9 + y0)
BCEW = EPC * JP             # 1024 channel-0 columns (16 exams x 64)
MW = 2 * EPC                # 32 diag mask columns
GROUPS = [2, 5, 5, 4]       # exams per DMA / matmul group
STARTS = [0, 2, 7, 12]
OUTW = PW + LWC + BCEW      # 1632 output columns

IMAGE_WEIGHT = 0.0736196319
EXAM_WEIGHTS = np.array([0.0736196319, 0.09202453988, 0.1042944785, 0.1042944785,
                         0.1877300613, 0.06257668712, 0.06257668712, 0.2346625767,
                         0.0782208589], dtype=np.float64)

_NC_CACHE = {}


def build_nc():
    nc = bacc.Bacc(trn_type="TRN2")
    f32 = mybir.dt.float32
    bf16 = mybir.dt.bfloat16
    fp8 = mybir.dt.float8e4
    DR = mybir.MatmulPerfMode.DoubleRow
    pc19 = nc.declare_dram_parameter("pc19", [NP, EPC, 2, PW], fp8,
                                     isOutput=False)
    lc19y = nc.declare_dram_parameter("lc19y", [NP, EPC, 2, LWC], fp8,
                                      isOutput=False)
    p0m = nc.declare_dram_parameter("p0m", [NP, BCEW + MW], bf16,
                                    isOutput=False)
    mask8 = nc.declare_dram_parameter("mask8", [NP, EPC, 2, MW], fp8,
                                      isOutput=False)
    out = nc.declare_dram_parameter("out", [MW, OUTW], f32, isOutput=True)

    with tile.TileContext(nc) as tc, ExitStack() as ctx:
        pool = ctx.enter_context(tc.tile_pool(name="main", bufs=1))
        psum = ctx.enter_context(tc.tile_pool(name="psum", bufs=1, space="PSUM"))

        # dedicated tiles (no recycling -> every DMA can be in flight)
        t_p0m = pool.tile([NP, BCEW + MW], bf16, tag="p0m")
        t_mask8 = pool.tile([NP, EPC, 2, MW], fp8, tag="mask8")
        Tp = [pool.tile([NP, n, 2, PW], fp8, tag=f"Tp{g}", name=f"Tp{g}")
              for g, n in enumerate(GROUPS)]
        Tl = [pool.tile([NP, n, 2, LWC], fp8, tag=f"Tl{g}", name=f"Tl{g}")
              for g, n in enumerate(GROUPS)]
        LP = pool.tile([NP, BCEW], bf16, tag="LP")
        LQ = pool.tile([NP, BCEW], bf16, tag="LQ")
        T1 = pool.tile([NP, BCEW], bf16, tag="T1")
        BCE = pool.tile([NP, BCEW], bf16, tag="BCE")
        OUT = pool.tile([MW, OUTW], f32, tag="OUT")

        # ring A (sync HWDGE): p0+diag mask first, then pred groups
        nc.sync.dma_start(out=t_p0m, in_=p0m[:, :])
        for g, n in enumerate(GROUPS):
            s = STARTS[g]
            nc.sync.dma_start(out=Tp[g], in_=pc19[:, s:s + n, :, :])
        # ring B (scalar HWDGE): DoubleRow masks, then label groups
        nc.scalar.dma_start(out=t_mask8, in_=mask8[:, :, :, :])
        for g, n in enumerate(GROUPS):
            s = STARTS[g]
            nc.scalar.dma_start(out=Tl[g], in_=lc19y[:, s:s + n, :, :])

        # image-path logs on ScalarE (contiguous bf16, ready early)
        p0v = t_p0m[:, 0:BCEW]
        nc.scalar.activation(out=LP, in_=p0v,
                             func=mybir.ActivationFunctionType.Ln)
        nc.scalar.activation(out=LQ, in_=p0v,
                             func=mybir.ActivationFunctionType.Ln,
                             bias=1.0, scale=-1.0)
        # bce = y0*(lp - lq) + lq  (= -image BCE), per label group
        for g, n in enumerate(GROUPS):
            s = STARTS[g]
            cs = slice(JP * s, JP * (s + n))
            y0v = Tl[g][:, :, :, PW:LWC]
            sh = dict(e=n, i=2, j=J2)
            nc.vector.tensor_sub(T1[:, cs], LP[:, cs], LQ[:, cs])
            nc.vector.tensor_mul(
                T1[:, cs].rearrange("p (e i j) -> p e i j", **sh),
                T1[:, cs].rearrange("p (e i j) -> p e i j", **sh), y0v)
            nc.vector.tensor_add(BCE[:, cs], T1[:, cs], LQ[:, cs])

        # psum accumulators
        Pp = psum.tile([MW, PW], f32, tag="Pp")
        Pl = psum.tile([MW, LWC], f32, tag="Pl")
        PB0 = psum.tile([MW, BCEW // 2], f32, tag="PB0")
        PB1 = psum.tile([MW, BCEW // 2], f32, tag="PB1")

        for g, n in enumerate(GROUPS):
            for eo in range(n):
                e = STARTS[g] + eo
                lhsT_e = t_mask8[:, e]
                st = dict(start=(e == 0), stop=(e == EPC - 1))
                nc.tensor.matmul(Pp, lhsT_e, Tp[g][:, eo], perf_mode=DR, **st)
                nc.tensor.matmul(Pl, lhsT_e, Tl[g][:, eo], perf_mode=DR, **st)
        maskd = t_p0m[:, BCEW:BCEW + MW]
        nc.tensor.matmul(PB0, maskd, BCE[:, 0:BCEW // 2])
        nc.tensor.matmul(PB1, maskd, BCE[:, BCEW // 2:BCEW])

        # raw psum -> SBUF (split across two engines), then DMA out
        nc.scalar.copy(OUT[:, 0:PW], Pp)
        nc.scalar.copy(OUT[:, PW + LWC:PW + LWC + BCEW // 2], PB0)
        nc.vector.tensor_copy(OUT[:, PW:PW + LWC], Pl)
        nc.vector.tensor_copy(OUT[:, PW + LWC + BCEW // 2:OUTW], PB1)
        nc.sync.dma_start(out=out[:, :], in_=OUT)
    nc.finalize()
    return nc


def _mask_tensors(lens):
    """Per-core DoubleRow lhsT [128,16,2,32] and diag [128,32] masks."""
    v_idx = np.arange(2 * NP).reshape(NP, 2)       # v = 2p + i
    p_idx = np.arange(NP)
    m8 = np.zeros((NP, EPC, 2, MW), np.float32)
    md = np.zeros((NP, MW), np.float32)
    for e, ln in enumerate(lens):
        P32 = int(ln) // J2
        m8[:, e, :, 2 * e] = (v_idx <= P32)
        m8[:, e, :, 2 * e + 1] = (v_idx < P32)
        P64 = int(ln) // JP
        md[:, 2 * e] = (p_idx <= P64)
        md[:, 2 * e + 1] = (p_idx < P64)
    return m8, md


def make_in_maps(pred, label, seq_lens):
    import ml_dtypes
    f8 = np.dtype(ml_dtypes.float8_e4m3fn)
    bf16np = mybir.dt.np(mybir.dt.bfloat16)
    in_maps = []
    for i in range(N_CORES):
        sl = slice(i * EPC, (i + 1) * EPC)
        r = pred[sl].reshape(EPC, NP, 2, J2, C)
        pc19 = np.ascontiguousarray(
            r[..., 1:].transpose(1, 0, 2, 3, 4)).reshape(NP, EPC, 2, PW)
        p0 = r[..., 0].reshape(EPC, NP, JP).transpose(1, 0, 2).reshape(NP, BCEW)
        rl = label[sl].reshape(EPC, NP, 2, J2, C)
        lc19y = np.concatenate(
            [rl[..., 1:].reshape(EPC, NP, 2, PW),
             rl[..., 0].reshape(EPC, NP, 2, J2)],
            axis=3).transpose(1, 0, 2, 3)
        m8, md = _mask_tensors(seq_lens[sl])
        p0m = np.concatenate([p0, md], axis=1)
        in_maps.append({
            "pc19": pc19.astype(f8),
            "lc19y": np.ascontiguousarray(lc19y).astype(f8),
            "p0m": p0m.astype(bf16np),
            "mask8": m8.astype(f8),
        })
    return in_maps


def finish(outs, seq_lens):
    """Host-side s/t fold + final combine from the 8 [32, 1632] dumps."""
    w = EXAM_WEIGHTS
    j32 = np.arange(J2)
    j64 = np.arange(JP)
    exam_loss = 0.0
    image_loss = 0.0
    tw_img = 0.0
    for i in range(N_CORES):
        O = outs[i].astype(np.float64)
        lens = seq_lens[i * EPC:(i + 1) * EPC].astype(np.float64)
        r32 = (lens % J2).astype(np.int64)
        s = (j32[None, :] < r32[:, None]).astype(np.float64)    # [16, 32]
        t = 1.0 - s
        r64 = (lens % JP).astype(np.int64)
        s6 = (j64[None, :] < r64[:, None]).astype(np.float64)   # [16, 64]
        t6 = 1.0 - s6
        A, Bp = O[0::2], O[1::2]                                # [16, 1632]
        Pa = A[:, 0:PW].reshape(EPC, J2, C9)
        Pb = Bp[:, 0:PW].reshape(EPC, J2, C9)
        predsum = np.einsum('ej,ejc->ec', s, Pa) + np.einsum('ej,ejc->ec', t, Pb)
        La = A[:, PW:PW + LWC]
        Lb = Bp[:, PW:PW + LWC]
        labsum = (np.einsum('ej,ejc->ec', s, La[:, 0:PW].reshape(EPC, J2, C9))
                  + np.einsum('ej,ejc->ec', t, Lb[:, 0:PW].reshape(EPC, J2, C9)))
        y0sum = (np.sum(s * La[:, PW:LWC], axis=1)
                 + np.sum(t * Lb[:, PW:LWC], axis=1))
        # bce: cols 608:1632, exam e owns cols 64e:64e+64 (l-order, 64 mask)
        Ba = A[:, PW + LWC:].reshape(EPC, EPC, JP)[np.arange(EPC), np.arange(EPC)]
        Bb = Bp[:, PW + LWC:].reshape(EPC, EPC, JP)[np.arange(EPC), np.arange(EPC)]
        bcesum = np.sum(s6 * Ba, axis=1) + np.sum(t6 * Bb, axis=1)

        pm = predsum / lens[:, None]
        ym = labsum / lens[:, None]
        exam_bce = -(ym * np.log(pm) + (1.0 - ym) * np.log(1.0 - pm))
        exam_loss += float(np.sum(exam_bce * w[None, :]))
        y0m = y0sum / lens
        imgw = IMAGE_WEIGHT * y0m
        image_loss += float(np.sum(-bcesum * imgw))
        tw_img += float(np.sum(imgw * lens))
    total_weights = B * float(np.sum(w)) + tw_img
    return np.float32((exam_loss + image_loss) / total_weights)


def kernel(pred, label, seq_lens):
    if "nc" not in _NC_CACHE:
        _NC_CACHE["nc"] = build_nc()
    nc = _NC_CACHE["nc"]
    in_maps = make_in_maps(np.asarray(pred), np.asarray(label),
                           np.asarray(seq_lens))
    res = run_bass_kernel_spmd(nc, in_maps, core_ids=list(range(N_CORES)))
    outs = [res.results[i]["out"] for i in range(N_CORES)]
    return finish(outs, np.asarray(seq_lens))


if __name__ == "__main__":
    rng = np.random.default_rng(0)
    pred = (rng.random((B, L, C), np.float32) * 0.98 + 0.01).astype(np.float32)
    label = (rng.random((B, L, C), np.float32) * 0.98 + 0.01).astype(np.float32)
    seq_lens = rng.integers(1, L + 1, size=(B,)).astype(np.int32)
    got = kernel(pred=pred, label=label, seq_lens=seq_lens)
    print("kernel:", got)


# revision 17
# speedup vs baseline: 2.1722x; 1.0996x over previous
"""RSNA loss kernel for Trainium2, SPMD across 8 NeuronCores.

Strategy (data-parallel over batch):
  - Shard B=128 exams -> 16 per core.
  - Host pre-splits each [8192, 10] exam into channels 1-9 + channel 0
    and quantizes: ch1-9 and label-ch0 (y0) to fp8e4m3, pred-ch0 (p0)
    to bf16 (rel tol is 2e-2; end-to-end error ~2.4e-5, verified on the
    fixed inputs). Layout is partition-major; partition p holds
    l in [64p, 64p+64), split as two interleaved 32-blocks (i, j):
      pc19  [128, 16, 2, 288] fp8   pred ch1-9, free = (i, j, c)
      lc19y [128, 16, 2, 320] fp8   label ch1-9 (cols 0:288) + y0
                                    (cols 288:320) per i-half
      p0m   [128, 1056] bf16        pred ch0 (cols 0:1024, l-order)
                                    + diag a/b mask (cols 1024:1056)
      mask8 [128, 16, 2, 32] fp8    per-exam DoubleRow lhsT (a/b at
                                    cols 2e/2e+1, rows v = 2p+i)
  - The seq_len mask over virtual rows v = l//32 is rank-2:
        mask[v,j] = a[v]*s[j] + b[v]*t[j],  a=[v <= len//32] etc.
    One fp8 DoubleRow matmul per exam per tensor (contraction over
    256 virtual rows, 2x throughput) accumulates a/b-weighted sums
    into psum rows 2e/2e+1; the tiny s/t fold over j happens on the
    HOST from the raw psum dump.
  - Image BCE: Ln(p0), Ln(1-p0) on ScalarE, bce = y0*(lp-lq)+lq on
    VectorE per label group, masked-summed by a bf16 diag a/b matmul
    (64-block mask); host folds s/t.
  - All loads ride the two hardware-DGE rings (sync + scalar engines),
    kicked up-front into dedicated tiles. First group is small so the
    PE pipeline starts early; ring loads are balanced.
  - Device outputs per core: raw psum [32, 1632]; host does the tiny
    final combine in f64.
"""
import numpy as np
from contextlib import ExitStack

import concourse.bass as bass
import concourse.bacc as bacc
import concourse.tile as tile
from concourse import mybir
from concourse.bass_utils import run_bass_kernel_spmd

N_CORES = 8
B, L, C = 128, 8192, 10
EPC = B // N_CORES          # exams per core = 16
JP = 64                     # l's per partition
NP = 128                    # partitions
C9 = C - 1                  # exam-path channels
J2 = JP // 2                # 32 l's per virtual row
PW = J2 * C9                # 288 pred cols per exam (per i-half)
LWC = PW + J2               # 320 label cols per exam (ch1-9 + y0)
BCEW = EPC * JP             # 1024 channel-0 columns (16 exams x 64)
MW = 2 * EPC                # 32 diag mask columns
GROUPS = [2, 5, 5, 4]       # exams per DMA / matmul group
STARTS = [0, 2, 7, 12]
OUTW = PW + LWC + BCEW      # 1632 output columns

IMAGE_WEIGHT = 0.0736196319
EXAM_WEIGHTS = np.array([0.0736196319, 0.09202453988, 0.1042944785, 0.1042944785,
                         0.1877300613, 0.06257668712, 0.06257668712, 0.2346625767,
                         0.0782208589], dtype=np.float64)

_NC_CACHE = {}


def build_nc():
    nc = bacc.Bacc(trn_type="TRN2")
    f32 = mybir.dt.float32
    bf16 = mybir.dt.bfloat16
    fp8 = mybir.dt.float8e4
    DR = mybir.MatmulPerfMode.DoubleRow
    data = nc.declare_dram_parameter("data", [NP, EPC, 2, PW + LWC], fp8,
                                     isOutput=False)
    p0m = nc.declare_dram_parameter("p0m", [NP, BCEW + MW], bf16,
                                    isOutput=False)
    mask8 = nc.declare_dram_parameter("mask8", [NP, EPC, 2, MW], fp8,
                                      isOutput=False)
    out = nc.declare_dram_parameter("out", [MW, OUTW], f32, isOutput=True)

    with tile.TileContext(nc) as tc, ExitStack() as ctx:
        pool = ctx.enter_context(tc.tile_pool(name="main", bufs=1))
        psum = ctx.enter_context(tc.tile_pool(name="psum", bufs=1, space="PSUM"))

        # dedicated tiles (no recycling -> every DMA can be in flight)
        t_p0m = pool.tile([NP, BCEW + MW], bf16, tag="p0m")
        t_mask8 = pool.tile([NP, EPC, 2, MW], fp8, tag="mask8")
        Td = [pool.tile([NP, n, 2, PW + LWC], fp8, tag=f"Td{g}", name=f"Td{g}")
              for g, n in enumerate(GROUPS)]
        LP = pool.tile([NP, BCEW], bf16, tag="LP")
        LQ = pool.tile([NP, BCEW], bf16, tag="LQ")
        T1 = pool.tile([NP, BCEW], bf16, tag="T1")
        BCE = pool.tile([NP, BCEW], bf16, tag="BCE")
        OUT = pool.tile([MW, OUTW], f32, tag="OUT")

        # balanced HWDGE rings: A (sync): p0m, groups 0+2; B (scalar):
        # DoubleRow masks, groups 1+3
        def kick(ring, g):
            s, n = STARTS[g], GROUPS[g]
            ring(out=Td[g], in_=data[:, s:s + n, :, :])
        nc.sync.dma_start(out=t_p0m, in_=p0m[:, :])
        kick(nc.sync.dma_start, 0)
        kick(nc.sync.dma_start, 2)
        nc.scalar.dma_start(out=t_mask8, in_=mask8[:, :, :, :])
        kick(nc.scalar.dma_start, 1)
        kick(nc.scalar.dma_start, 3)

        # image-path logs on ScalarE (contiguous bf16, ready early)
        p0v = t_p0m[:, 0:BCEW]
        nc.scalar.activation(out=LP, in_=p0v,
                             func=mybir.ActivationFunctionType.Ln)
        nc.scalar.activation(out=LQ, in_=p0v,
                             func=mybir.ActivationFunctionType.Ln,
                             bias=1.0, scale=-1.0)
        # bce = y0*(lp - lq) + lq  (= -image BCE), per group, on GpSimd
        # (keeps VectorE free for the psum dump; a scheduling pass would
        # otherwise hoist the psum copy ahead and block this chain)
        for g, n in enumerate(GROUPS):
            s = STARTS[g]
            cs = slice(JP * s, JP * (s + n))
            y0v = Td[g][:, :, :, 2 * PW:PW + LWC]
            sh = dict(e=n, i=2, j=J2)
            nc.gpsimd.tensor_sub(T1[:, cs], LP[:, cs], LQ[:, cs])
            nc.gpsimd.tensor_mul(
                T1[:, cs].rearrange("p (e i j) -> p e i j", **sh),
                T1[:, cs].rearrange("p (e i j) -> p e i j", **sh), y0v)
            nc.gpsimd.tensor_add(BCE[:, cs], T1[:, cs], LQ[:, cs])

        # psum accumulators
        Pp = psum.tile([MW, PW], f32, tag="Pp")
        Pl = psum.tile([MW, LWC], f32, tag="Pl")
        PB0 = psum.tile([MW, BCEW // 2], f32, tag="PB0")
        PB1 = psum.tile([MW, BCEW // 2], f32, tag="PB1")

        for g, n in enumerate(GROUPS):
            for eo in range(n):
                e = STARTS[g] + eo
                lhsT_e = t_mask8[:, e]
                st = dict(start=(e == 0), stop=(e == EPC - 1))
                nc.tensor.matmul(Pp, lhsT_e, Td[g][:, eo, :, 0:PW],
                                 perf_mode=DR, **st)
                nc.tensor.matmul(Pl, lhsT_e, Td[g][:, eo, :, PW:PW + LWC],
                                 perf_mode=DR, **st)
        maskd = t_p0m[:, BCEW:BCEW + MW]
        nc.tensor.matmul(PB0, maskd, BCE[:, 0:BCEW // 2])
        nc.tensor.matmul(PB1, maskd, BCE[:, BCEW // 2:BCEW])

        # raw psum -> SBUF (split across two engines), then DMA out
        nc.scalar.copy(OUT[:, 0:PW], Pp)
        nc.scalar.copy(OUT[:, PW + LWC:PW + LWC + BCEW // 2], PB0)
        nc.vector.tensor_copy(OUT[:, PW:PW + LWC], Pl)
        nc.vector.tensor_copy(OUT[:, PW + LWC + BCEW // 2:OUTW], PB1)
        nc.sync.dma_start(out=out[:, :], in_=OUT)
    nc.finalize()
    return nc


def _mask_tensors(lens):
    """Per-core DoubleRow lhsT [128,16,2,32] and diag [128,32] masks."""
    v_idx = np.arange(2 * NP).reshape(NP, 2)       # v = 2p + i
    p_idx = np.arange(NP)
    m8 = np.zeros((NP, EPC, 2, MW), np.float32)
    md = np.zeros((NP, MW), np.float32)
    for e, ln in enumerate(lens):
        P32 = int(ln) // J2
        m8[:, e, :, 2 * e] = (v_idx <= P32)
        m8[:, e, :, 2 * e + 1] = (v_idx < P32)
        P64 = int(ln) // JP
        md[:, 2 * e] = (p_idx <= P64)
        md[:, 2 * e + 1] = (p_idx < P64)
    return m8, md


def make_in_maps(pred, label, seq_lens):
    import ml_dtypes
    f8 = np.dtype(ml_dtypes.float8_e4m3fn)
    bf16np = mybir.dt.np(mybir.dt.bfloat16)
    in_maps = []
    for i in range(N_CORES):
        sl = slice(i * EPC, (i + 1) * EPC)
        r = pred[sl].reshape(EPC, NP, 2, J2, C)
        p0 = r[..., 0].reshape(EPC, NP, JP).transpose(1, 0, 2).reshape(NP, BCEW)
        rl = label[sl].reshape(EPC, NP, 2, J2, C)
        # per exam-half: [pred ch1-9 (288) | label ch1-9 (288) | y0 (32)]
        d = np.concatenate(
            [r[..., 1:].reshape(EPC, NP, 2, PW),
             rl[..., 1:].reshape(EPC, NP, 2, PW),
             rl[..., 0].reshape(EPC, NP, 2, J2)],
            axis=3).transpose(1, 0, 2, 3)
        m8, md = _mask_tensors(seq_lens[sl])
        p0m = np.concatenate([p0, md], axis=1)
        in_maps.append({
            "data": np.ascontiguousarray(d).astype(f8),
            "p0m": p0m.astype(bf16np),
            "mask8": m8.astype(f8),
        })
    return in_maps


def finish(outs, seq_lens):
    """Host-side s/t fold + final combine from the 8 [32, 1632] dumps."""
    w = EXAM_WEIGHTS
    j32 = np.arange(J2)
    j64 = np.arange(JP)
    exam_loss = 0.0
    image_loss = 0.0
    tw_img = 0.0
    for i in range(N_CORES):
        O = outs[i].astype(np.float64)
        lens = seq_lens[i * EPC:(i + 1) * EPC].astype(np.float64)
        r32 = (lens % J2).astype(np.int64)
        s = (j32[None, :] < r32[:, None]).astype(np.float64)    # [16, 32]
        t = 1.0 - s
        r64 = (lens % JP).astype(np.int64)
        s6 = (j64[None, :] < r64[:, None]).astype(np.float64)   # [16, 64]
        t6 = 1.0 - s6
        A, Bp = O[0::2], O[1::2]                                # [16, 1632]
        Pa = A[:, 0:PW].reshape(EPC, J2, C9)
        Pb = Bp[:, 0:PW].reshape(EPC, J2, C9)
        predsum = np.einsum('ej,ejc->ec', s, Pa) + np.einsum('ej,ejc->ec', t, Pb)
        La = A[:, PW:PW + LWC]
        Lb = Bp[:, PW:PW + LWC]
        labsum = (np.einsum('ej,ejc->ec', s, La[:, 0:PW].reshape(EPC, J2, C9))
                  + np.einsum('ej,ejc->ec', t, Lb[:, 0:PW].reshape(EPC, J2, C9)))
        y0sum = (np.sum(s * La[:, PW:LWC], axis=1)
                 + np.sum(t * Lb[:, PW:LWC], axis=1))
        # bce: cols 608:1632, exam e owns cols 64e:64e+64 (l-order, 64 mask)
        Ba = A[:, PW + LWC:].reshape(EPC, EPC, JP)[np.arange(EPC), np.arange(EPC)]
        Bb = Bp[:, PW + LWC:].reshape(EPC, EPC, JP)[np.arange(EPC), np.arange(EPC)]
        bcesum = np.sum(s6 * Ba, axis=1) + np.sum(t6 * Bb, axis=1)

        pm = predsum / lens[:, None]
        ym = labsum / lens[:, None]
        exam_bce = -(ym * np.log(pm) + (1.0 - ym) * np.log(1.0 - pm))
        exam_loss += float(np.sum(exam_bce * w[None, :]))
        y0m = y0sum / lens
        imgw = IMAGE_WEIGHT * y0m
        image_loss += float(np.sum(-bcesum * imgw))
        tw_img += float(np.sum(imgw * lens))
    total_weights = B * float(np.sum(w)) + tw_img
    return np.float32((exam_loss + image_loss) / total_weights)


def kernel(pred, label, seq_lens):
    if "nc" not in _NC_CACHE:
        _NC_CACHE["nc"] = build_nc()
    nc = _NC_CACHE["nc"]
    in_maps = make_in_maps(np.asarray(pred), np.asarray(label),
                           np.asarray(seq_lens))
    res = run_bass_kernel_spmd(nc, in_maps, core_ids=list(range(N_CORES)))
    outs = [res.results[i]["out"] for i in range(N_CORES)]
    return finish(outs, np.asarray(seq_lens))


if __name__ == "__main__":
    rng = np.random.default_rng(0)
    pred = (rng.random((B, L, C), np.float32) * 0.98 + 0.01).astype(np.float32)
    label = (rng.random((B, L, C), np.float32) * 0.98 + 0.01).astype(np.float32)
    seq_lens = rng.integers(1, L + 1, size=(B,)).astype(np.int32)
    got = kernel(pred=pred, label=label, seq_lens=seq_lens)
    print("kernel:", got)


# revision 19
# speedup vs baseline: 2.2012x; 1.0134x over previous
"""RSNA loss kernel for Trainium2, SPMD across 8 NeuronCores.

Strategy (data-parallel over batch):
  - Shard B=128 exams -> 16 per core.
  - Host pre-splits each [8192, 10] exam into channels 1-9 + channel 0
    and quantizes: ch1-9 and label-ch0 (y0) to fp8e4m3, pred-ch0 (p0)
    to bf16 (rel tol is 2e-2; end-to-end error ~2.4e-5, verified on the
    fixed inputs). Layout is partition-major; partition p holds
    l in [64p, 64p+64), split as two interleaved 32-blocks (i, j):
      pc19  [128, 16, 2, 288] fp8   pred ch1-9, free = (i, j, c)
      lc19y [128, 16, 2, 320] fp8   label ch1-9 (cols 0:288) + y0
                                    (cols 288:320) per i-half
      p0m   [128, 1056] bf16        pred ch0 (cols 0:1024, l-order)
                                    + diag a/b mask (cols 1024:1056)
      mask8 [128, 16, 2, 32] fp8    per-exam DoubleRow lhsT (a/b at
                                    cols 2e/2e+1, rows v = 2p+i)
  - The seq_len mask over virtual rows v = l//32 is rank-2:
        mask[v,j] = a[v]*s[j] + b[v]*t[j],  a=[v <= len//32] etc.
    One fp8 DoubleRow matmul per exam per tensor (contraction over
    256 virtual rows, 2x throughput) accumulates a/b-weighted sums
    into psum rows 2e/2e+1; the tiny s/t fold over j happens on the
    HOST from the raw psum dump.
  - Image BCE: Ln(p0), Ln(1-p0) on ScalarE, bce = y0*(lp-lq)+lq on
    VectorE per label group, masked-summed by a bf16 diag a/b matmul
    (64-block mask); host folds s/t.
  - All loads ride the two hardware-DGE rings (sync + scalar engines),
    kicked up-front into dedicated tiles. First group is small so the
    PE pipeline starts early; ring loads are balanced.
  - Device outputs per core: raw psum [32, 1632]; host does the tiny
    final combine in f64.
"""
import numpy as np
from contextlib import ExitStack

import concourse.bass as bass
import concourse.bacc as bacc
import concourse.tile as tile
from concourse import mybir
from concourse.bass_utils import run_bass_kernel_spmd

N_CORES = 8
B, L, C = 128, 8192, 10
EPC = B // N_CORES          # exams per core = 16
JP = 64                     # l's per partition
NP = 128                    # partitions
C9 = C - 1                  # exam-path channels
J2 = JP // 2                # 32 l's per virtual row
PW = J2 * C9                # 288 pred cols per exam (per i-half)
LWC = PW + J2               # 320 label cols per exam (ch1-9 + y0)
BCEW = EPC * JP             # 1024 channel-0 columns (16 exams x 64)
MW = 2 * EPC                # 32 diag mask columns
GROUPS = [2, 2, 3, 3, 3, 3]  # exams per DMA / matmul group
STARTS = [0, 2, 4, 7, 10, 13]
OUTW = PW + LWC + BCEW      # 1632 output columns

IMAGE_WEIGHT = 0.0736196319
EXAM_WEIGHTS = np.array([0.0736196319, 0.09202453988, 0.1042944785, 0.1042944785,
                         0.1877300613, 0.06257668712, 0.06257668712, 0.2346625767,
                         0.0782208589], dtype=np.float64)

_NC_CACHE = {}


def build_nc():
    nc = bacc.Bacc(trn_type="TRN2")
    f32 = mybir.dt.float32
    bf16 = mybir.dt.bfloat16
    fp8 = mybir.dt.float8e4
    DR = mybir.MatmulPerfMode.DoubleRow
    data = nc.declare_dram_parameter("data", [NP, EPC, 2, PW + LWC], fp8,
                                     isOutput=False)
    p0m = nc.declare_dram_parameter("p0m", [NP, BCEW + MW], bf16,
                                    isOutput=False)
    mask8 = nc.declare_dram_parameter("mask8", [NP, EPC, 2, MW], fp8,
                                      isOutput=False)
    out = nc.declare_dram_parameter("out", [MW, OUTW], f32, isOutput=True)

    with tile.TileContext(nc) as tc, ExitStack() as ctx:
        pool = ctx.enter_context(tc.tile_pool(name="main", bufs=1))
        psum = ctx.enter_context(tc.tile_pool(name="psum", bufs=1, space="PSUM"))

        # dedicated tiles (no recycling -> every DMA can be in flight)
        t_p0m = pool.tile([NP, BCEW + MW], bf16, tag="p0m")
        t_mask8 = pool.tile([NP, EPC, 2, MW], fp8, tag="mask8")
        Td = [pool.tile([NP, n, 2, PW + LWC], fp8, tag=f"Td{g}", name=f"Td{g}")
              for g, n in enumerate(GROUPS)]
        LP = pool.tile([NP, BCEW], bf16, tag="LP")
        LQ = pool.tile([NP, BCEW], bf16, tag="LQ")
        T1 = pool.tile([NP, BCEW], bf16, tag="T1")
        BCE = pool.tile([NP, BCEW], bf16, tag="BCE")
        OUT = pool.tile([MW, OUTW], f32, tag="OUT")

        # balanced HWDGE rings: A (sync): even groups + p0m; B (scalar):
        # DoubleRow masks + odd groups. First groups are small so the PE
        # pipeline starts early.
        def kick(ring, g):
            s, n = STARTS[g], GROUPS[g]
            ring(out=Td[g], in_=data[:, s:s + n, :, :])
        kick(nc.sync.dma_start, 0)
        nc.sync.dma_start(out=t_p0m, in_=p0m[:, :])
        kick(nc.sync.dma_start, 2)
        kick(nc.sync.dma_start, 4)
        nc.scalar.dma_start(out=t_mask8, in_=mask8[:, :, :, :])
        kick(nc.scalar.dma_start, 1)
        kick(nc.scalar.dma_start, 3)
        kick(nc.scalar.dma_start, 5)

        # image-path logs on ScalarE (contiguous bf16, ready early)
        p0v = t_p0m[:, 0:BCEW]
        nc.scalar.activation(out=LP, in_=p0v,
                             func=mybir.ActivationFunctionType.Ln)
        nc.scalar.activation(out=LQ, in_=p0v,
                             func=mybir.ActivationFunctionType.Ln,
                             bias=1.0, scale=-1.0)
        # bce = y0*(lp - lq) + lq  (= -image BCE), per group, on GpSimd
        # (keeps VectorE free for the psum dump; a scheduling pass would
        # otherwise hoist the psum copy ahead and block this chain)
        for g, n in enumerate(GROUPS):
            s = STARTS[g]
            cs = slice(JP * s, JP * (s + n))
            y0v = Td[g][:, :, :, 2 * PW:PW + LWC]
            sh = dict(e=n, i=2, j=J2)
            nc.gpsimd.tensor_sub(T1[:, cs], LP[:, cs], LQ[:, cs])
            nc.gpsimd.tensor_mul(
                T1[:, cs].rearrange("p (e i j) -> p e i j", **sh),
                T1[:, cs].rearrange("p (e i j) -> p e i j", **sh), y0v)
            nc.gpsimd.tensor_add(BCE[:, cs], T1[:, cs], LQ[:, cs])

        # psum accumulators
        Pp = psum.tile([MW, PW], f32, tag="Pp")
        Pl = psum.tile([MW, LWC], f32, tag="Pl")
        PB0 = psum.tile([MW, BCEW // 2], f32, tag="PB0")
        PB1 = psum.tile([MW, BCEW // 2], f32, tag="PB1")

        for g, n in enumerate(GROUPS):
            for eo in range(n):
                e = STARTS[g] + eo
                lhsT_e = t_mask8[:, e]
                st = dict(start=(e == 0), stop=(e == EPC - 1))
                nc.tensor.matmul(Pp, lhsT_e, Td[g][:, eo, :, 0:PW],
                                 perf_mode=DR, **st)
                nc.tensor.matmul(Pl, lhsT_e, Td[g][:, eo, :, PW:PW + LWC],
                                 perf_mode=DR, **st)
        maskd = t_p0m[:, BCEW:BCEW + MW]
        nc.tensor.matmul(PB0, maskd, BCE[:, 0:BCEW // 2])
        nc.tensor.matmul(PB1, maskd, BCE[:, BCEW // 2:BCEW])

        # raw psum -> SBUF (split across two engines), then DMA out
        nc.scalar.copy(OUT[:, 0:PW], Pp)
        nc.scalar.copy(OUT[:, PW + LWC:PW + LWC + BCEW // 2], PB0)
        nc.vector.tensor_copy(OUT[:, PW:PW + LWC], Pl)
        nc.vector.tensor_copy(OUT[:, PW + LWC + BCEW // 2:OUTW], PB1)
        nc.sync.dma_start(out=out[:, :], in_=OUT)
    nc.finalize()
    return nc


def _mask_tensors(lens):
    """Per-core DoubleRow lhsT [128,16,2,32] and diag [128,32] masks."""
    v_idx = np.arange(2 * NP).reshape(NP, 2)       # v = 2p + i
    p_idx = np.arange(NP)
    m8 = np.zeros((NP, EPC, 2, MW), np.float32)
    md = np.zeros((NP, MW), np.float32)
    for e, ln in enumerate(lens):
        P32 = int(ln) // J2
        m8[:, e, :, 2 * e] = (v_idx <= P32)
        m8[:, e, :, 2 * e + 1] = (v_idx < P32)
        P64 = int(ln) // JP
        md[:, 2 * e] = (p_idx <= P64)
        md[:, 2 * e + 1] = (p_idx < P64)
    return m8, md


def make_in_maps(pred, label, seq_lens):
    import ml_dtypes
    f8 = np.dtype(ml_dtypes.float8_e4m3fn)
    bf16np = mybir.dt.np(mybir.dt.bfloat16)
    in_maps = []
    for i in range(N_CORES):
        sl = slice(i * EPC, (i + 1) * EPC)
        r = pred[sl].reshape(EPC, NP, 2, J2, C)
        p0 = r[..., 0].reshape(EPC, NP, JP).transpose(1, 0, 2).reshape(NP, BCEW)
        rl = label[sl].reshape(EPC, NP, 2, J2, C)
        # per exam-half: [pred ch1-9 (288) | label ch1-9 (288) | y0 (32)]
        d = np.concatenate(
            [r[..., 1:].reshape(EPC, NP, 2, PW),
             rl[..., 1:].reshape(EPC, NP, 2, PW),
             rl[..., 0].reshape(EPC, NP, 2, J2)],
            axis=3).transpose(1, 0, 2, 3)
        m8, md = _mask_tensors(seq_lens[sl])
        p0m = np.concatenate([p0, md], axis=1)
        in_maps.append({
            "data": np.ascontiguousarray(d).astype(f8),
            "p0m": p0m.astype(bf16np),
            "mask8": m8.astype(f8),
        })
    return in_maps


def finish(outs, seq_lens):
    """Host-side s/t fold + final combine from the 8 [32, 1632] dumps."""
    w = EXAM_WEIGHTS
    j32 = np.arange(J2)
    j64 = np.arange(JP)
    exam_loss = 0.0
    image_loss = 0.0
    tw_img = 0.0
    for i in range(N_CORES):
        O = outs[i].astype(np.float64)
        lens = seq_lens[i * EPC:(i + 1) * EPC].astype(np.float64)
        r32 = (lens % J2).astype(np.int64)
        s = (j32[None, :] < r32[:, None]).astype(np.float64)    # [16, 32]
        t = 1.0 - s
        r64 = (lens % JP).astype(np.int64)
        s6 = (j64[None, :] < r64[:, None]).astype(np.float64)   # [16, 64]
        t6 = 1.0 - s6
        A, Bp = O[0::2], O[1::2]                                # [16, 1632]
        Pa = A[:, 0:PW].reshape(EPC, J2, C9)
        Pb = Bp[:, 0:PW].reshape(EPC, J2, C9)
        predsum = np.einsum('ej,ejc->ec', s, Pa) + np.einsum('ej,ejc->ec', t, Pb)
        La = A[:, PW:PW + LWC]
        Lb = Bp[:, PW:PW + LWC]
        labsum = (np.einsum('ej,ejc->ec', s, La[:, 0:PW].reshape(EPC, J2, C9))
                  + np.einsum('ej,ejc->ec', t, Lb[:, 0:PW].reshape(EPC, J2, C9)))
        y0sum = (np.sum(s * La[:, PW:LWC], axis=1)
                 + np.sum(t * Lb[:, PW:LWC], axis=1))
        # bce: cols 608:1632, exam e owns cols 64e:64e+64 (l-order, 64 mask)
        Ba = A[:, PW + LWC:].reshape(EPC, EPC, JP)[np.arange(EPC), np.arange(EPC)]
        Bb = Bp[:, PW + LWC:].reshape(EPC, EPC, JP)[np.arange(EPC), np.arange(EPC)]
        bcesum = np.sum(s6 * Ba, axis=1) + np.sum(t6 * Bb, axis=1)

        pm = predsum / lens[:, None]
        ym = labsum / lens[:, None]
        exam_bce = -(ym * np.log(pm) + (1.0 - ym) * np.log(1.0 - pm))
        exam_loss += float(np.sum(exam_bce * w[None, :]))
        y0m = y0sum / lens
        imgw = IMAGE_WEIGHT * y0m
        image_loss += float(np.sum(-bcesum * imgw))
        tw_img += float(np.sum(imgw * lens))
    total_weights = B * float(np.sum(w)) + tw_img
    return np.float32((exam_loss + image_loss) / total_weights)


def kernel(pred, label, seq_lens):
    if "nc" not in _NC_CACHE:
        _NC_CACHE["nc"] = build_nc()
    nc = _NC_CACHE["nc"]
    in_maps = make_in_maps(np.asarray(pred), np.asarray(label),
                           np.asarray(seq_lens))
    res = run_bass_kernel_spmd(nc, in_maps, core_ids=list(range(N_CORES)))
    outs = [res.results[i]["out"] for i in range(N_CORES)]
    return finish(outs, np.asarray(seq_lens))


if __name__ == "__main__":
    rng = np.random.default_rng(0)
    pred = (rng.random((B, L, C), np.float32) * 0.98 + 0.01).astype(np.float32)
    label = (rng.random((B, L, C), np.float32) * 0.98 + 0.01).astype(np.float32)
    seq_lens = rng.integers(1, L + 1, size=(B,)).astype(np.int32)
    got = kernel(pred=pred, label=label, seq_lens=seq_lens)
    print("kernel:", got)
